# revision 16
# baseline (speedup 1.0000x reference)
"""GCF (graph collaborative filtering) message passing on 8 Trainium2 cores.

Sharding: nodes split contiguously, core c owns rows [c*12500, (c+1)*12500).
Per layer:
  SpMM  - edges bucketed by (col-range r, dest 256-row block B); per 128-edge
          chunk: bulk dma_gather of source feature rows (f32r), one-hot
          O[e, row] = vals[e]*(rowl[e]==row) in a single tensor_scalar op,
          PE accumulates Lx^T += Xg^T @ O per bucket in PSUM (fp32r matmuls);
          bucket results accumulate into an SBUF Lx^T buffer (FT_out reused).
  Dense - transposed layout: y^T = Wlin^T@(Lx+F)^T + Wint^T@(Lx*F)^T in PSUM,
          fused bias+leaky-relu on ACT, row norms via ones-matmul, reciprocal,
          K=1 broadcast matmul, multiply; PE transposes write the normal-layout
          shard for the AllGather.
  AllGather shares the new features with all cores.
Final: batch split 2048/core, item rows bucketed by col-range; per concat level
  dma_gather u/i rows, multiply+reduce, sum over levels; host inverts the
  bucketing permutation.

Index trick: dma_gather indices are int16, so the 100k-row feature tables are
addressed through 4 range slices of 32768 rows (idx = row - r*32768).
"""

import os

import numpy as np

import concourse.bacc as bacc
import concourse.mybir as mybir
import concourse.tile as tile
from concourse.bass import ts
from concourse.bass_utils import run_bass_kernel_spmd
from concourse.masks import make_identity

NUM_USERS = 30000
NUM_ITEMS = 70000
N = 100000
D = 128
NL = 3
BATCH = 16384
NCORE = 8
SHARD = N // NCORE            # 12500
RB = 256                      # dest rows per block
NB = (SHARD + RB - 1) // RB   # 49 blocks/core
RS = 32768                    # range size (int16 index window)
NR = 4                        # ranges
GMAX = 24                     # max chunks per dma_gather call
BSH = BATCH // NCORE          # 2048
EPS = 1e-12
SLOPE = 0.01

f32 = mybir.dt.float32
f32r = mybir.dt.float32r
i16 = mybir.dt.int16

_cache = {}

# debug bisection: 0=consts only, 1=+spmm, 2=+dense, 3=+allgather(1 layer),
# 4=+all layers, 5=full (default)
STAGE = int(os.environ.get("KSTAGE", "5"))
NLAYERS = int(os.environ.get("KNL", str(NL)))
KGROUPS = int(os.environ.get("KGROUPS", "100000"))
KANY = os.environ.get("KANY", "1") == "1"
KREPEAT = int(os.environ.get("KREPEAT", "1"))


def _build(meta):
    (NCH, k2, first_r, groups, chunk_info, NFB, fin_bounds) = meta
    nc = bacc.Bacc(num_devices=NCORE)

    feat0 = nc.dram_tensor("feat0", [N, D], f32, kind="ExternalInput")
    f0t = nc.dram_tensor("f0t", [D, SHARD], f32, kind="ExternalInput")
    eidx = nc.dram_tensor("eidx", [128, NCH * 8], i16, kind="ExternalInput")
    erow = nc.dram_tensor("erow", [128, NCH], f32, kind="ExternalInput")
    evals = nc.dram_tensor("evals", [128, NCH], f32, kind="ExternalInput")
    wlin = nc.dram_tensor("wlin", [D, NL * D], f32, kind="ExternalInput")
    wint = nc.dram_tensor("wint", [D, NL * D], f32, kind="ExternalInput")
    biasc = nc.dram_tensor("biasc", [D, NL], f32, kind="ExternalInput")
    uidx = nc.dram_tensor("uidx", [128, NFB * 8], i16, kind="ExternalInput")
    iidx = nc.dram_tensor("iidx", [128, NFB * 8], i16, kind="ExternalInput")
    score = nc.dram_tensor("score", [128, NFB], f32, kind="ExternalOutput")

    add = mybir.AluOpType.add
    mult = mybir.AluOpType.mult
    is_equal = mybir.AluOpType.is_equal
    maxop = mybir.AluOpType.max
    AF = mybir.ActivationFunctionType

    with tile.TileContext(nc) as tc:
        with (
            tc.tile_pool(name="const", bufs=1) as cp,
            tc.tile_pool(name="ft", bufs=1) as ftp,
            tc.tile_pool(name="g", bufs=2) as gp,
            tc.tile_pool(name="fin", bufs=1) as fp_,
            tc.tile_pool(name="ot", bufs=4) as otp,
            tc.tile_pool(name="sb", bufs=3) as sbp,
            tc.tile_pool(name="plx", bufs=2, space="PSUM") as plx,
            tc.tile_pool(name="py", bufs=2, space="PSUM") as pyp,
            tc.tile_pool(name="pn", bufs=1, space="PSUM") as pnp,
            tc.tile_pool(name="pbc", bufs=1, space="PSUM") as pbc,
            tc.tile_pool(name="ptp", bufs=2, space="PSUM") as ptp,
            tc.tile_pool(name="dram", bufs=1, space="DRAM") as dp,
        ):
            # ---------- constants ----------
            iota = cp.tile([128, RB], f32)
            nc.gpsimd.iota(iota[:], pattern=[[1, RB]], base=0,
                           channel_multiplier=0,
                           allow_small_or_imprecise_dtypes=True)
            ident = cp.tile([128, 128], f32)
            make_identity(nc, ident[:])
            ones_f = cp.tile([128, 1], f32)
            nc.vector.memset(ones_f[:], 1.0)
            ones_r = cp.tile([128, 1], f32r)
            nc.vector.tensor_copy(ones_r[:], ones_f[:])
            onesrow_f = cp.tile([1, 128], f32)
            nc.vector.memset(onesrow_f[:], 1.0)
            onesrow_r = cp.tile([1, 128], f32r)
            nc.vector.tensor_copy(onesrow_r[:], onesrow_f[:])

            wlin_f = cp.tile([128, NL * 128], f32)
            nc.sync.dma_start(out=wlin_f[:], in_=wlin[:])
            wlin_r = cp.tile([128, NL * 128], f32r)
            nc.vector.tensor_copy(wlin_r[:], wlin_f[:])
            wint_f = cp.tile([128, NL * 128], f32)
            nc.sync.dma_start(out=wint_f[:], in_=wint[:])
            wint_r = cp.tile([128, NL * 128], f32r)
            nc.vector.tensor_copy(wint_r[:], wint_f[:])
            bias_sb = cp.tile([128, NL], f32)
            nc.sync.dma_start(out=bias_sb[:], in_=biasc[:])

            eidx_sb = cp.tile([128, NCH * 8], i16)
            nc.sync.dma_start(out=eidx_sb[:], in_=eidx[:])
            erow_sb = cp.tile([128, NCH], f32)
            nc.sync.dma_start(out=erow_sb[:], in_=erow[:])
            evals_sb = cp.tile([128, NCH], f32)
            nc.sync.dma_start(out=evals_sb[:], in_=evals[:])
            uidx_sb = cp.tile([128, NFB * 8], i16)
            nc.sync.dma_start(out=uidx_sb[:], in_=uidx[:])
            iidx_sb = cp.tile([128, NFB * 8], i16)
            nc.sync.dma_start(out=iidx_sb[:], in_=iidx[:])

            fta = ftp.tile([128, NB * RB], f32, tag="fta")
            ftb = ftp.tile([128, NB * RB], f32, tag="ftb")
            nc.vector.memset(fta[:, SHARD:], 0.0)
            nc.sync.dma_start(out=fta[:, :SHARD], in_=f0t[:])

            fshard = dp.tile([SHARD, D], f32)
            agreps = [
                [dp.tile([N, D], f32, name=f"ag{r}_{i}", tag=f"ag{r}_{i}",
                         addr_space="Shared")
                 for i in range(NL)]
                for r in range(KREPEAT)
            ]
            ags = agreps[0]

            n_layers = NLAYERS if STAGE >= 4 else (1 if STAGE >= 1 else 0)
            for rep_l in range(KREPEAT * n_layers):
                rep, l = divmod(rep_l, n_layers)
                ftin = fta if l % 2 == 0 else ftb
                ftout = ftb if l % 2 == 0 else fta
                ags = agreps[rep]
                src = feat0 if l == 0 else ags[l - 1]

                # ---------- SpMM sweep (range-major chunk order) ----------
                cur_lx = None
                for gi, (gr, q0, gn) in enumerate(groups[:KGROUPS]):
                    gt = gp.tile([128, GMAX * 128], f32r, tag="g",
                                 name=f"g{rep}_{l}_{gi}")
                    nc.gpsimd.dma_gather(
                        gt[:, :gn * 128].rearrange("p (c d) -> p c d", d=128),
                        src[gr * RS:, :].bitcast(f32r),
                        eidx_sb[:, q0 * 8:(q0 + gn) * 8],
                        gn * 128, gn * 128, 128,
                        single_packet=False,
                    )
                    for q in range(q0, q0 + gn):
                        r_q, b_q, is_first, is_last, loc = chunk_info[q]
                        o = otp.tile([128, RB], f32r, tag="o",
                                     name=f"o{rep}_{l}_{q}")
                        ts_eng = nc.gpsimd if (KANY and q % 3 == 2) \
                            else nc.vector
                        ts_eng.tensor_scalar(
                            out=o[:], in0=iota[:],
                            scalar1=erow_sb[:, q:q + 1],
                            scalar2=evals_sb[:, q:q + 1],
                            op0=is_equal, op1=mult,
                        )
                        if is_first:
                            cur_lx = plx.tile([128, RB], f32, tag="lx",
                                              name=f"lx{rep}_{l}_{q}")
                        nc.tensor.matmul(
                            out=cur_lx[:],
                            lhsT=gt[:, ts(loc, 128)],
                            rhs=o[:],
                            start=is_first, stop=is_last,
                        )
                        if is_last:
                            dst = ftout[:, ts(b_q, RB)]
                            if first_r[b_q] == r_q:
                                nc.vector.tensor_copy(dst, cur_lx[:])
                            else:
                                nc.vector.tensor_tensor(
                                    out=dst, in0=dst, in1=cur_lx[:], op=add)

                # ---------- dense phase ----------
                for b in range(NB if STAGE >= 2 else 0):
                    lxs = ftout[:, ts(b, RB)]
                    fin_ = ftin[:, ts(b, RB)]
                    pre1 = sbp.tile([128, RB], f32r, tag="pre1")
                    nc.vector.tensor_tensor(out=pre1[:], in0=lxs, in1=fin_,
                                            op=add)
                    pre2 = sbp.tile([128, RB], f32r, tag="pre2")
                    nc.vector.tensor_tensor(out=pre2[:], in0=lxs, in1=fin_,
                                            op=mult)
                    y = pyp.tile([128, RB], f32, tag="y")
                    nc.tensor.matmul(out=y[:], lhsT=wlin_r[:, ts(l, 128)],
                                     rhs=pre1[:], start=True, stop=False)
                    nc.tensor.matmul(out=y[:], lhsT=wint_r[:, ts(l, 128)],
                                     rhs=pre2[:], start=False, stop=True)
                    ya = sbp.tile([128, RB], f32, tag="ya")
                    nc.scalar.activation(out=ya[:], in_=y[:], func=AF.Lrelu,
                                         bias=bias_sb[:, l:l + 1], scale=1.0,
                                         alpha=SLOPE)
                    sq = sbp.tile([128, RB], f32r, tag="sq")
                    nc.vector.tensor_tensor(out=sq[:], in0=ya[:], in1=ya[:],
                                            op=mult)
                    nsq = pnp.tile([1, RB], f32, tag="nsq")
                    nc.tensor.matmul(out=nsq[:], lhsT=ones_r[:], rhs=sq[:],
                                     start=True, stop=True)
                    rt = sbp.tile([1, RB], f32, tag="rt")
                    nc.scalar.activation(out=rt[:], in_=nsq[:], func=AF.Sqrt)
                    rtm = sbp.tile([1, RB], f32, tag="rtm")
                    nc.vector.tensor_scalar(out=rtm[:], in0=rt[:],
                                            scalar1=EPS, scalar2=None,
                                            op0=maxop)
                    inv = sbp.tile([1, RB], f32r, tag="inv")
                    with nc.allow_low_precision(reason="f32r broadcast input"):
                        nc.vector.reciprocal(inv[:], rtm[:])
                    bc = pbc.tile([128, RB], f32, tag="bc")
                    nc.tensor.matmul(out=bc[:], lhsT=onesrow_r[:], rhs=inv[:],
                                     start=True, stop=True)
                    nc.vector.tensor_tensor(out=ftout[:, ts(b, RB)],
                                            in0=ya[:], in1=bc[:], op=mult)
                    # transpose to normal layout, write shard
                    for h in range(2):
                        r0 = b * RB + h * 128
                        nr = min(128, SHARD - r0)
                        if nr <= 0:
                            break
                        tp = ptp.tile([128, 128], f32, tag="tp",
                                      name=f"tp{rep}_{l}_{b}_{h}")
                        nc.tensor.transpose(
                            out=tp[:], in_=ftout[:, r0:r0 + 128],
                            identity=ident[:])
                        cpo = sbp.tile([128, 128], f32, tag="cpo")
                        nc.vector.tensor_copy(cpo[:], tp[:])
                        nc.sync.dma_start(out=fshard[r0:r0 + nr, :],
                                          in_=cpo[:nr, :])

                if STAGE >= 3:
                    nc.gpsimd.collective_compute(
                        "AllGather", mybir.AluOpType.bypass,
                        replica_groups=[list(range(NCORE))],
                        ins=[fshard.opt()], outs=[ags[l].opt()],
                    )

            # ---------- final gather + dot ----------
            acc = cp.tile([128, NFB], f32)
            if STAGE < 5:
                nc.vector.memset(acc[:], 0.0)
            fin_list = (([feat0] + ags) if STAGE >= 5 else []) * KREPEAT
            for li, srcf in enumerate(fin_list):
                ug = fp_.tile([128, NFB * 128], f32, tag="ug",
                              name=f"ug{li}")
                nc.gpsimd.dma_gather(
                    ug[:].rearrange("p (c d) -> p c d", d=128),
                    srcf[:],
                    uidx_sb[:],
                    NFB * 128, NFB * 128, 128,
                    single_packet=False,
                )
                ig = fp_.tile([128, NFB * 128], f32, tag="ig",
                              name=f"ig{li}")
                for (rr, c0, cn) in fin_bounds:
                    nc.gpsimd.dma_gather(
                        ig[:, c0 * 128:(c0 + cn) * 128].rearrange(
                            "p (c d) -> p c d", d=128),
                        srcf[rr * RS:, :],
                        iidx_sb[:, c0 * 8:(c0 + cn) * 8],
                        cn * 128, cn * 128, 128,
                        single_packet=False,
                    )
                nc.vector.tensor_tensor(out=ug[:], in0=ug[:], in1=ig[:],
                                        op=mult)
                sc = sbp.tile([128, NFB], f32, tag="sc")
                nc.vector.tensor_reduce(
                    out=sc[:],
                    in_=ug[:].rearrange("p (c d) -> p c d", d=128),
                    axis=mybir.AxisListType.X, op=add)
                if li % (NL + 1) == 0:
                    nc.vector.tensor_copy(acc[:], sc[:])
                else:
                    nc.vector.tensor_tensor(out=acc[:], in0=acc[:],
                                            in1=sc[:], op=add)
            nc.sync.dma_start(out=score[:], in_=acc[:])

    nc.compile()
    return nc


def _pack_inputs(userIdx, itemIdx, rows, cols, vals, uEmbd, iEmbd,
                 Wlin, blin, Wint, bint):
    rows = np.asarray(rows, dtype=np.int64)
    cols = np.asarray(cols, dtype=np.int64)
    vals = np.asarray(vals, dtype=np.float32)
    userIdx = np.asarray(userIdx, dtype=np.int64)
    itemIdx = np.asarray(itemIdx, dtype=np.int64)

    feat0 = np.ascontiguousarray(
        np.concatenate([np.asarray(uEmbd, np.float32),
                        np.asarray(iEmbd, np.float32)], axis=0))

    # ---- edge bucketing: (core, range, block) ----
    core = rows // SHARD
    local = rows - core * SHARD
    blk = local // RB
    rowl = (local - blk * RB).astype(np.float32)
    rng = cols // RS
    col_local = (cols - rng * RS).astype(np.int16)

    bkey = ((core * NR + rng) * NB + blk).astype(np.int64)
    order = np.argsort(bkey, kind="stable")
    bkey_s = bkey[order]
    counts = np.bincount(bkey_s, minlength=NCORE * NR * NB)
    counts = counts.reshape(NCORE, NR, NB)
    k2 = np.ceil(counts.max(axis=0) / 128).astype(np.int64)  # [NR, NB]

    # chunk layout (range-major), shared by all cores
    chunk_base = np.zeros((NR, NB), dtype=np.int64)
    nch = 0
    chunk_info = []          # per chunk: (r, b, is_first, is_last, loc)
    groups = []              # (r, q0, n)
    first_r = [None] * NB
    for r in range(NR):
        for b in range(NB):
            k = int(k2[r, b])
            if k == 0:
                continue
            if first_r[b] is None:
                first_r[b] = r
            chunk_base[r, b] = nch
            for j in range(k):
                chunk_info.append([r, b, j == 0, j == k - 1, 0])
            nch += k
    NCH = nch
    # gather groups: consecutive chunks, same range, <= GMAX
    q = 0
    while q < NCH:
        r = chunk_info[q][0]
        n = 1
        while (q + n < NCH and n < GMAX and chunk_info[q + n][0] == r):
            n += 1
        groups.append((r, q, n))
        for j in range(n):
            chunk_info[q + j][4] = j
        q += n
    chunk_info = [tuple(x) for x in chunk_info]

    # ---- scatter edges into slots ----
    starts = np.zeros(NCORE * NR * NB, dtype=np.int64)
    np.cumsum(counts.reshape(-1)[:-1], out=starts[1:])
    pos = np.arange(len(bkey_s), dtype=np.int64) - starts[bkey_s]
    core_s = core[order]
    rng_s = rng[order]
    blk_s = blk[order]
    q_of_edge = chunk_base[rng_s, blk_s] + pos // 128
    p_of_edge = pos % 128

    eidx_arr = np.zeros((NCORE, 16, NCH * 8), dtype=np.int16)
    erow_arr = np.zeros((NCORE, 128, NCH), dtype=np.float32)
    eval_arr = np.zeros((NCORE, 128, NCH), dtype=np.float32)
    eidx_arr[core_s, p_of_edge % 16, q_of_edge * 8 + p_of_edge // 16] = \
        col_local[order]
    erow_arr[core_s, p_of_edge, q_of_edge] = rowl[order]
    eval_arr[core_s, p_of_edge, q_of_edge] = vals[order]

    # ---- weights ----
    wlin_h = np.ascontiguousarray(
        np.asarray(Wlin, np.float32).transpose(1, 0, 2).reshape(D, NL * D))
    wint_h = np.ascontiguousarray(
        np.asarray(Wint, np.float32).transpose(1, 0, 2).reshape(D, NL * D))
    biasc = np.ascontiguousarray(
        (np.asarray(blin, np.float32) + np.asarray(bint, np.float32)).T)

    # ---- final stage: bucket item rows by range ----
    irow = itemIdx + NUM_USERS
    ir = irow // RS
    # per-core bucketing with shared padded sizes
    nfb_counts = np.zeros((NCORE, NR), dtype=np.int64)
    perms = []
    for c in range(NCORE):
        sl = slice(c * BSH, (c + 1) * BSH)
        o = np.argsort(ir[sl], kind="stable")
        perms.append(o)
        nfb_counts[c] = np.bincount(ir[sl][o], minlength=NR)
    bucket_chunks = np.ceil(nfb_counts.max(axis=0) / 128).astype(np.int64)
    # drop empty buckets, build (range, chunk0, nchunks)
    fin_bounds = []
    c0 = 0
    for r in range(NR):
        n = int(bucket_chunks[r])
        if n == 0:
            continue
        fin_bounds.append((r, c0, n))
        c0 += n
    NFB = c0

    uidx_arr = np.zeros((NCORE, 16, NFB * 8), dtype=np.int16)
    iidx_arr = np.zeros((NCORE, 16, NFB * 8), dtype=np.int16)
    inv_perm = np.full((NCORE, NFB * 128), -1, dtype=np.int64)
    for c in range(NCORE):
        sl = slice(c * BSH, (c + 1) * BSH)
        o = perms[c]
        u_s = userIdx[sl][o]
        i_s = irow[sl][o]
        r_s = ir[sl][o]
        # slot for j-th sorted elem: bucket r -> slots [b0*128 ...]
        jpos = np.zeros(BSH, dtype=np.int64)
        for (r, b0, nchk) in fin_bounds:
            m = r_s == r
            jpos[m] = b0 * 128 + np.arange(int(m.sum()))
        uidx_arr[c, jpos % 16, (jpos // 128) * 8 + (jpos % 128) // 16] = \
            u_s.astype(np.int16)
        iidx_arr[c, jpos % 16, (jpos // 128) * 8 + (jpos % 128) // 16] = \
            (i_s - r_s * RS).astype(np.int16)
        inv_perm[c, jpos] = np.arange(c * BSH, (c + 1) * BSH)[o]

    meta = (NCH, k2, tuple(first_r), tuple(groups), tuple(chunk_info),
            NFB, tuple(fin_bounds))

    in_maps = []
    for c in range(NCORE):
        f0t = np.ascontiguousarray(feat0[c * SHARD:(c + 1) * SHARD].T)
        in_maps.append({
            "feat0": feat0,
            "f0t": f0t,
            "eidx": np.ascontiguousarray(np.tile(eidx_arr[c], (8, 1))),
            "erow": np.ascontiguousarray(erow_arr[c]),
            "evals": np.ascontiguousarray(eval_arr[c]),
            "wlin": wlin_h,
            "wint": wint_h,
            "biasc": biasc,
            "uidx": np.ascontiguousarray(np.tile(uidx_arr[c], (8, 1))),
            "iidx": np.ascontiguousarray(np.tile(iidx_arr[c], (8, 1))),
        })
    return meta, in_maps, inv_perm


def kernel(**inputs) -> np.ndarray:
    meta, in_maps, inv_perm = _pack_inputs(**inputs)
    key = (meta[0], meta[5], meta[3], meta[4], meta[6], tuple(meta[2]))
    if key not in _cache:
        _cache[key] = _build(meta)
    nc = _cache[key]
    res = run_bass_kernel_spmd(nc, in_maps, list(range(NCORE)))
    out = np.empty(BATCH, dtype=np.float32)
    NFB = meta[5]
    for c in range(NCORE):
        sc = res.results[c]["score"]  # [128, NFB]
        # slot j -> sc[j % 128, j // 128]
        vals_j = sc[np.arange(NFB * 128) % 128, np.arange(NFB * 128) // 128]
        valid = inv_perm[c] >= 0
        out[inv_perm[c][valid]] = vals_j[valid]
    return out


# revision 17
# speedup vs baseline: 1.3137x; 1.3137x over previous
"""GCF (graph collaborative filtering) message passing on 8 Trainium2 cores.

Sharding: nodes split contiguously, core c owns rows [c*12500, (c+1)*12500).
Per layer:
  SpMM  - edges bucketed by (col-range r, dest 256-row block B); per 128-edge
          chunk: bulk dma_gather of source feature rows (f32r), one-hot
          O[e, row] = vals[e]*(rowl[e]==row) in a single tensor_scalar op,
          PE accumulates Lx^T += Xg^T @ O per bucket in PSUM (fp32r matmuls);
          bucket results accumulate into an SBUF Lx^T buffer (FT_out reused).
  Dense - transposed layout: y^T = Wlin^T@(Lx+F)^T + Wint^T@(Lx*F)^T in PSUM,
          fused bias+leaky-relu on ACT, row norms via ones-matmul, reciprocal,
          K=1 broadcast matmul, multiply; PE transposes write the normal-layout
          shard for the AllGather.
  AllGather shares the new features with all cores.
Final: batch split 2048/core, item rows bucketed by col-range; per concat level
  dma_gather u/i rows, multiply+reduce, sum over levels; host inverts the
  bucketing permutation.

Index trick: dma_gather indices are int16, so the 100k-row feature tables are
addressed through 4 range slices of 32768 rows (idx = row - r*32768).
"""

import os

import numpy as np

import concourse.bacc as bacc
import concourse.mybir as mybir
import concourse.tile as tile
from concourse.bass import ts
from concourse.bass_utils import run_bass_kernel_spmd
from concourse.masks import make_identity

NUM_USERS = 30000
NUM_ITEMS = 70000
N = 100000
D = 128
NL = 3
BATCH = 16384
NCORE = 8
SHARD = N // NCORE            # 12500
RB = 256                      # dest rows per block
NB = (SHARD + RB - 1) // RB   # 49 blocks/core
RS = 32768                    # range size (int16 index window)
NR = 4                        # ranges
GMAX = 24                     # max chunks per dma_gather call
BSH = BATCH // NCORE          # 2048
EPS = 1e-12
SLOPE = 0.01

f32 = mybir.dt.float32
f32r = mybir.dt.float32r
i16 = mybir.dt.int16

_cache = {}

# debug bisection: 0=consts only, 1=+spmm, 2=+dense, 3=+allgather(1 layer),
# 4=+all layers, 5=full (default)
STAGE = int(os.environ.get("KSTAGE", "5"))
NLAYERS = int(os.environ.get("KNL", str(NL)))
KGROUPS = int(os.environ.get("KGROUPS", "100000"))
KANY = os.environ.get("KANY", "1") == "1"
KREPEAT = int(os.environ.get("KREPEAT", "1"))
KONLY = os.environ.get("KONLY", "")


def _build(meta):
    (NCH, k2, first_r, groups, chunk_info, NFB, fin_bounds) = meta
    nc = bacc.Bacc(num_devices=NCORE)

    feat0 = nc.dram_tensor("feat0", [N, D], f32, kind="ExternalInput")
    f0t = nc.dram_tensor("f0t", [D, SHARD], f32, kind="ExternalInput")
    eidx = nc.dram_tensor("eidx", [128, NCH * 8], i16, kind="ExternalInput")
    erow = nc.dram_tensor("erow", [128, NCH], f32, kind="ExternalInput")
    evals = nc.dram_tensor("evals", [128, NCH], f32, kind="ExternalInput")
    wlin = nc.dram_tensor("wlin", [D, NL * D], f32, kind="ExternalInput")
    wint = nc.dram_tensor("wint", [D, NL * D], f32, kind="ExternalInput")
    biasc = nc.dram_tensor("biasc", [D, NL], f32, kind="ExternalInput")
    uidx = nc.dram_tensor("uidx", [128, NFB * 8], i16, kind="ExternalInput")
    iidx = nc.dram_tensor("iidx", [128, NFB * 8], i16, kind="ExternalInput")
    score = nc.dram_tensor("score", [128, NFB], f32, kind="ExternalOutput")

    add = mybir.AluOpType.add
    mult = mybir.AluOpType.mult
    is_equal = mybir.AluOpType.is_equal
    maxop = mybir.AluOpType.max
    AF = mybir.ActivationFunctionType

    with tile.TileContext(nc) as tc:
        with (
            tc.tile_pool(name="const", bufs=1) as cp,
            tc.tile_pool(name="ft", bufs=1) as ftp,
            tc.tile_pool(name="g", bufs=2) as gp,
            tc.tile_pool(name="fin", bufs=1) as fp_,
            tc.tile_pool(name="ot", bufs=4) as otp,
            tc.tile_pool(name="sb", bufs=3) as sbp,
            tc.tile_pool(name="plx", bufs=2, space="PSUM") as plx,
            tc.tile_pool(name="py", bufs=2, space="PSUM") as pyp,
            tc.tile_pool(name="pn", bufs=1, space="PSUM") as pnp,
            tc.tile_pool(name="pbc", bufs=1, space="PSUM") as pbc,
            tc.tile_pool(name="ptp", bufs=2, space="PSUM") as ptp,
            tc.tile_pool(name="dram", bufs=1, space="DRAM") as dp,
        ):
            # ---------- constants ----------
            iota = cp.tile([128, RB], f32)
            nc.gpsimd.iota(iota[:], pattern=[[1, RB]], base=0,
                           channel_multiplier=0,
                           allow_small_or_imprecise_dtypes=True)
            ident = cp.tile([128, 128], f32)
            make_identity(nc, ident[:])
            ones_f = cp.tile([128, 1], f32)
            nc.vector.memset(ones_f[:], 1.0)
            ones_r = cp.tile([128, 1], f32r)
            nc.vector.tensor_copy(ones_r[:], ones_f[:])
            onesrow_f = cp.tile([1, 128], f32)
            nc.vector.memset(onesrow_f[:], 1.0)
            onesrow_r = cp.tile([1, 128], f32r)
            nc.vector.tensor_copy(onesrow_r[:], onesrow_f[:])

            wlin_f = cp.tile([128, NL * 128], f32)
            nc.sync.dma_start(out=wlin_f[:], in_=wlin[:])
            wlin_r = cp.tile([128, NL * 128], f32r)
            nc.vector.tensor_copy(wlin_r[:], wlin_f[:])
            wint_f = cp.tile([128, NL * 128], f32)
            nc.sync.dma_start(out=wint_f[:], in_=wint[:])
            wint_r = cp.tile([128, NL * 128], f32r)
            nc.vector.tensor_copy(wint_r[:], wint_f[:])
            bias_sb = cp.tile([128, NL], f32)
            nc.sync.dma_start(out=bias_sb[:], in_=biasc[:])

            eidx_sb = cp.tile([128, NCH * 8], i16)
            nc.sync.dma_start(out=eidx_sb[:], in_=eidx[:])
            erow_sb = cp.tile([128, NCH], f32)
            nc.sync.dma_start(out=erow_sb[:], in_=erow[:])
            evals_sb = cp.tile([128, NCH], f32)
            nc.sync.dma_start(out=evals_sb[:], in_=evals[:])
            uidx_sb = cp.tile([128, NFB * 8], i16)
            nc.sync.dma_start(out=uidx_sb[:], in_=uidx[:])
            iidx_sb = cp.tile([128, NFB * 8], i16)
            nc.sync.dma_start(out=iidx_sb[:], in_=iidx[:])

            fta = ftp.tile([128, NB * RB], f32, tag="fta")
            ftb = ftp.tile([128, NB * RB], f32, tag="ftb")
            nc.vector.memset(fta[:, SHARD:], 0.0)
            nc.sync.dma_start(out=fta[:, :SHARD], in_=f0t[:])

            fshard = dp.tile([SHARD, D], f32)
            agreps = [
                [dp.tile([N, D], f32, name=f"ag{r}_{i}", tag=f"ag{r}_{i}",
                         addr_space="Shared")
                 for i in range(NL)]
                for r in range(KREPEAT)
            ]
            ags = agreps[0]

            n_layers = NLAYERS if STAGE >= 4 else (1 if STAGE >= 1 else 0)
            for rep_l in range(KREPEAT * n_layers):
                rep, l = divmod(rep_l, n_layers)
                ftin = fta if l % 2 == 0 else ftb
                ftout = ftb if l % 2 == 0 else fta
                ags = agreps[rep]
                src = feat0 if l == 0 else ags[l - 1]

                # ---------- SpMM sweep (range-major chunk order) ----------
                cur_lx = None
                for gi, (gr, q0, gn) in enumerate(groups[:KGROUPS]):
                    gt = gp.tile([128, GMAX * 128], f32r, tag="g",
                                 name=f"g{rep}_{l}_{gi}")
                    nc.gpsimd.dma_gather(
                        gt[:, :gn * 128].rearrange("p (c d) -> p c d", d=128),
                        src[gr * RS:, :].bitcast(f32r),
                        eidx_sb[:, q0 * 8:(q0 + gn) * 8],
                        gn * 128, gn * 128, 128,
                        single_packet=False,
                    )
                    if KONLY == "gather":
                        continue
                    for q in range(q0, q0 + gn):
                        r_q, b_q, is_first, is_last, loc = chunk_info[q]
                        o = otp.tile([128, RB], f32r, tag="o",
                                     name=f"o{rep}_{l}_{q}")
                        ts_eng = nc.gpsimd if (KANY and q % 3 == 2) \
                            else nc.vector
                        ts_eng.tensor_scalar(
                            out=o[:], in0=iota[:],
                            scalar1=erow_sb[:, q:q + 1],
                            scalar2=evals_sb[:, q:q + 1],
                            op0=is_equal, op1=mult,
                        )
                        if is_first:
                            cur_lx = plx.tile([128, RB], f32, tag="lx",
                                              name=f"lx{rep}_{l}_{q}")
                        nc.tensor.matmul(
                            out=cur_lx[:],
                            lhsT=gt[:, ts(loc, 128)],
                            rhs=o[:],
                            start=is_first, stop=is_last,
                        )
                        if is_last:
                            dst = ftout[:, ts(b_q, RB)]
                            if first_r[b_q] == r_q:
                                nc.vector.tensor_copy(dst, cur_lx[:])
                            else:
                                nc.vector.tensor_tensor(
                                    out=dst, in0=dst, in1=cur_lx[:], op=add)

                # ---------- dense phase ----------
                for b in range(NB if (STAGE >= 2 and not KONLY) else 0):
                    lxs = ftout[:, ts(b, RB)]
                    fin_ = ftin[:, ts(b, RB)]
                    pre1 = sbp.tile([128, RB], f32r, tag="pre1")
                    nc.vector.tensor_tensor(out=pre1[:], in0=lxs, in1=fin_,
                                            op=add)
                    pre2 = sbp.tile([128, RB], f32r, tag="pre2")
                    nc.vector.tensor_tensor(out=pre2[:], in0=lxs, in1=fin_,
                                            op=mult)
                    y = pyp.tile([128, RB], f32, tag="y")
                    nc.tensor.matmul(out=y[:], lhsT=wlin_r[:, ts(l, 128)],
                                     rhs=pre1[:], start=True, stop=False)
                    nc.tensor.matmul(out=y[:], lhsT=wint_r[:, ts(l, 128)],
                                     rhs=pre2[:], start=False, stop=True)
                    ya = sbp.tile([128, RB], f32, tag="ya")
                    nc.scalar.activation(out=ya[:], in_=y[:], func=AF.Lrelu,
                                         bias=bias_sb[:, l:l + 1], scale=1.0,
                                         alpha=SLOPE)
                    sq = sbp.tile([128, RB], f32r, tag="sq")
                    nc.vector.tensor_tensor(out=sq[:], in0=ya[:], in1=ya[:],
                                            op=mult)
                    nsq = pnp.tile([1, RB], f32, tag="nsq")
                    nc.tensor.matmul(out=nsq[:], lhsT=ones_r[:], rhs=sq[:],
                                     start=True, stop=True)
                    rt = sbp.tile([1, RB], f32, tag="rt")
                    nc.scalar.activation(out=rt[:], in_=nsq[:], func=AF.Sqrt)
                    rtm = sbp.tile([1, RB], f32, tag="rtm")
                    nc.vector.tensor_scalar(out=rtm[:], in0=rt[:],
                                            scalar1=EPS, scalar2=None,
                                            op0=maxop)
                    inv = sbp.tile([1, RB], f32r, tag="inv")
                    with nc.allow_low_precision(reason="f32r broadcast input"):
                        nc.vector.reciprocal(inv[:], rtm[:])
                    bc = pbc.tile([128, RB], f32, tag="bc")
                    nc.tensor.matmul(out=bc[:], lhsT=onesrow_r[:], rhs=inv[:],
                                     start=True, stop=True)
                    nc.vector.tensor_tensor(out=ftout[:, ts(b, RB)],
                                            in0=ya[:], in1=bc[:], op=mult)
                    # transpose to normal layout, write shard
                    for h in range(2):
                        r0 = b * RB + h * 128
                        nr = min(128, SHARD - r0)
                        if nr <= 0:
                            break
                        tp = ptp.tile([128, 128], f32, tag="tp",
                                      name=f"tp{rep}_{l}_{b}_{h}")
                        nc.tensor.transpose(
                            out=tp[:], in_=ftout[:, r0:r0 + 128],
                            identity=ident[:])
                        cpo = sbp.tile([128, 128], f32, tag="cpo")
                        nc.vector.tensor_copy(cpo[:], tp[:])
                        nc.sync.dma_start(out=fshard[r0:r0 + nr, :],
                                          in_=cpo[:nr, :])

                if STAGE >= 3:
                    nc.gpsimd.collective_compute(
                        "AllGather", mybir.AluOpType.bypass,
                        replica_groups=[list(range(NCORE))],
                        ins=[fshard.opt()], outs=[ags[l].opt()],
                    )

            # ---------- final gather + dot ----------
            acc = cp.tile([128, NFB], f32)
            if STAGE < 5:
                nc.vector.memset(acc[:], 0.0)
            fin_list = (([feat0] + ags) if STAGE >= 5 else []) * KREPEAT
            for li, srcf in enumerate(fin_list):
                ug = fp_.tile([128, NFB * 128], f32, tag="ug",
                              name=f"ug{li}")
                nc.gpsimd.dma_gather(
                    ug[:].rearrange("p (c d) -> p c d", d=128),
                    srcf[:],
                    uidx_sb[:],
                    NFB * 128, NFB * 128, 128,
                    single_packet=False,
                )
                ig = fp_.tile([128, NFB * 128], f32, tag="ig",
                              name=f"ig{li}")
                for (rr, c0, cn) in fin_bounds:
                    nc.gpsimd.dma_gather(
                        ig[:, c0 * 128:(c0 + cn) * 128].rearrange(
                            "p (c d) -> p c d", d=128),
                        srcf[rr * RS:, :],
                        iidx_sb[:, c0 * 8:(c0 + cn) * 8],
                        cn * 128, cn * 128, 128,
                        single_packet=False,
                    )
                nc.vector.tensor_tensor(out=ug[:], in0=ug[:], in1=ig[:],
                                        op=mult)
                sc = sbp.tile([128, NFB], f32, tag="sc")
                nc.vector.tensor_reduce(
                    out=sc[:],
                    in_=ug[:].rearrange("p (c d) -> p c d", d=128),
                    axis=mybir.AxisListType.X, op=add)
                if li % (NL + 1) == 0:
                    nc.vector.tensor_copy(acc[:], sc[:])
                else:
                    nc.vector.tensor_tensor(out=acc[:], in0=acc[:],
                                            in1=sc[:], op=add)
            nc.sync.dma_start(out=score[:], in_=acc[:])

    nc.compile()
    return nc


def _pack_inputs(userIdx, itemIdx, rows, cols, vals, uEmbd, iEmbd,
                 Wlin, blin, Wint, bint):
    rows = np.asarray(rows, dtype=np.int64)
    cols = np.asarray(cols, dtype=np.int64)
    vals = np.asarray(vals, dtype=np.float32)
    userIdx = np.asarray(userIdx, dtype=np.int64)
    itemIdx = np.asarray(itemIdx, dtype=np.int64)

    feat0 = np.ascontiguousarray(
        np.concatenate([np.asarray(uEmbd, np.float32),
                        np.asarray(iEmbd, np.float32)], axis=0))

    # ---- edge bucketing: (core, range, block) ----
    core = rows // SHARD
    local = rows - core * SHARD
    blk = local // RB
    rowl = (local - blk * RB).astype(np.float32)
    rng = cols // RS
    col_local = (cols - rng * RS).astype(np.int16)

    bkey = ((core * NR + rng) * NB + blk).astype(np.int64)
    order = np.argsort(bkey, kind="stable")
    bkey_s = bkey[order]
    counts = np.bincount(bkey_s, minlength=NCORE * NR * NB)
    counts = counts.reshape(NCORE, NR, NB)
    k2 = np.ceil(counts.max(axis=0) / 128).astype(np.int64)  # [NR, NB]

    # chunk layout (range-major), shared by all cores
    chunk_base = np.zeros((NR, NB), dtype=np.int64)
    nch = 0
    chunk_info = []          # per chunk: (r, b, is_first, is_last, loc)
    groups = []              # (r, q0, n)
    first_r = [None] * NB
    for r in range(NR):
        for b in range(NB):
            k = int(k2[r, b])
            if k == 0:
                continue
            if first_r[b] is None:
                first_r[b] = r
            chunk_base[r, b] = nch
            for j in range(k):
                chunk_info.append([r, b, j == 0, j == k - 1, 0])
            nch += k
    NCH = nch
    # gather groups: consecutive chunks, same range, <= GMAX
    q = 0
    while q < NCH:
        r = chunk_info[q][0]
        n = 1
        while (q + n < NCH and n < GMAX and chunk_info[q + n][0] == r):
            n += 1
        groups.append((r, q, n))
        for j in range(n):
            chunk_info[q + j][4] = j
        q += n
    chunk_info = [tuple(x) for x in chunk_info]

    # ---- scatter edges into slots ----
    starts = np.zeros(NCORE * NR * NB, dtype=np.int64)
    np.cumsum(counts.reshape(-1)[:-1], out=starts[1:])
    pos = np.arange(len(bkey_s), dtype=np.int64) - starts[bkey_s]
    core_s = core[order]
    rng_s = rng[order]
    blk_s = blk[order]
    q_of_edge = chunk_base[rng_s, blk_s] + pos // 128
    p_of_edge = pos % 128

    eidx_arr = np.zeros((NCORE, 16, NCH * 8), dtype=np.int16)
    erow_arr = np.zeros((NCORE, 128, NCH), dtype=np.float32)
    eval_arr = np.zeros((NCORE, 128, NCH), dtype=np.float32)
    eidx_arr[core_s, p_of_edge % 16, q_of_edge * 8 + p_of_edge // 16] = \
        col_local[order]
    erow_arr[core_s, p_of_edge, q_of_edge] = rowl[order]
    eval_arr[core_s, p_of_edge, q_of_edge] = vals[order]

    # ---- weights ----
    wlin_h = np.ascontiguousarray(
        np.asarray(Wlin, np.float32).transpose(1, 0, 2).reshape(D, NL * D))
    wint_h = np.ascontiguousarray(
        np.asarray(Wint, np.float32).transpose(1, 0, 2).reshape(D, NL * D))
    biasc = np.ascontiguousarray(
        (np.asarray(blin, np.float32) + np.asarray(bint, np.float32)).T)

    # ---- final stage: bucket item rows by range ----
    irow = itemIdx + NUM_USERS
    ir = irow // RS
    # per-core bucketing with shared padded sizes
    nfb_counts = np.zeros((NCORE, NR), dtype=np.int64)
    perms = []
    for c in range(NCORE):
        sl = slice(c * BSH, (c + 1) * BSH)
        o = np.argsort(ir[sl], kind="stable")
        perms.append(o)
        nfb_counts[c] = np.bincount(ir[sl][o], minlength=NR)
    bucket_chunks = np.ceil(nfb_counts.max(axis=0) / 128).astype(np.int64)
    # drop empty buckets, build (range, chunk0, nchunks)
    fin_bounds = []
    c0 = 0
    for r in range(NR):
        n = int(bucket_chunks[r])
        if n == 0:
            continue
        fin_bounds.append((r, c0, n))
        c0 += n
    NFB = c0

    uidx_arr = np.zeros((NCORE, 16, NFB * 8), dtype=np.int16)
    iidx_arr = np.zeros((NCORE, 16, NFB * 8), dtype=np.int16)
    inv_perm = np.full((NCORE, NFB * 128), -1, dtype=np.int64)
    for c in range(NCORE):
        sl = slice(c * BSH, (c + 1) * BSH)
        o = perms[c]
        u_s = userIdx[sl][o]
        i_s = irow[sl][o]
        r_s = ir[sl][o]
        # slot for j-th sorted elem: bucket r -> slots [b0*128 ...]
        jpos = np.zeros(BSH, dtype=np.int64)
        for (r, b0, nchk) in fin_bounds:
            m = r_s == r
            jpos[m] = b0 * 128 + np.arange(int(m.sum()))
        uidx_arr[c, jpos % 16, (jpos // 128) * 8 + (jpos % 128) // 16] = \
            u_s.astype(np.int16)
        iidx_arr[c, jpos % 16, (jpos // 128) * 8 + (jpos % 128) // 16] = \
            (i_s - r_s * RS).astype(np.int16)
        inv_perm[c, jpos] = np.arange(c * BSH, (c + 1) * BSH)[o]

    meta = (NCH, k2, tuple(first_r), tuple(groups), tuple(chunk_info),
            NFB, tuple(fin_bounds))

    in_maps = []
    for c in range(NCORE):
        f0t = np.ascontiguousarray(feat0[c * SHARD:(c + 1) * SHARD].T)
        in_maps.append({
            "feat0": feat0,
            "f0t": f0t,
            "eidx": np.ascontiguousarray(np.tile(eidx_arr[c], (8, 1))),
            "erow": np.ascontiguousarray(erow_arr[c]),
            "evals": np.ascontiguousarray(eval_arr[c]),
            "wlin": wlin_h,
            "wint": wint_h,
            "biasc": biasc,
            "uidx": np.ascontiguousarray(np.tile(uidx_arr[c], (8, 1))),
            "iidx": np.ascontiguousarray(np.tile(iidx_arr[c], (8, 1))),
        })
    return meta, in_maps, inv_perm


def kernel(**inputs) -> np.ndarray:
    meta, in_maps, inv_perm = _pack_inputs(**inputs)
    key = (meta[0], meta[5], meta[3], meta[4], meta[6], tuple(meta[2]))
    if key not in _cache:
        _cache[key] = _build(meta)
    nc = _cache[key]
    res = run_bass_kernel_spmd(nc, in_maps, list(range(NCORE)))
    out = np.empty(BATCH, dtype=np.float32)
    NFB = meta[5]
    for c in range(NCORE):
        sc = res.results[c]["score"]  # [128, NFB]
        # slot j -> sc[j % 128, j // 128]
        vals_j = sc[np.arange(NFB * 128) % 128, np.arange(NFB * 128) // 128]
        valid = inv_perm[c] >= 0
        out[inv_perm[c][valid]] = vals_j[valid]
    return out


# revision 19
# speedup vs baseline: 50.9634x; 38.7938x over previous
"""GCF (graph collaborative filtering) message passing on 8 Trainium2 cores.

Sharding: nodes split contiguously, core c owns rows [c*12500, (c+1)*12500).
Per layer:
  SpMM  - edges bucketed by (col-range r, dest 256-row block B); per 128-edge
          chunk: bulk dma_gather of source feature rows (f32r), one-hot
          O[e, row] = vals[e]*(rowl[e]==row) in a single tensor_scalar op,
          PE accumulates Lx^T += Xg^T @ O per bucket in PSUM (fp32r matmuls);
          bucket results accumulate into an SBUF Lx^T buffer (FT_out reused).
  Dense - transposed layout: y^T = Wlin^T@(Lx+F)^T + Wint^T@(Lx*F)^T in PSUM,
          fused bias+leaky-relu on ACT, row norms via ones-matmul, reciprocal,
          K=1 broadcast matmul, multiply; PE transposes write the normal-layout
          shard for the AllGather.
  AllGather shares the new features with all cores.
Final: batch split 2048/core, item rows bucketed by col-range; per concat level
  dma_gather u/i rows, multiply+reduce, sum over levels; host inverts the
  bucketing permutation.

Index trick: dma_gather indices are int16, so the 100k-row feature tables are
addressed through 4 range slices of 32768 rows (idx = row - r*32768).
"""

import os

import numpy as np

import concourse.bacc as bacc
import concourse.mybir as mybir
import concourse.tile as tile
from concourse.bass import ts
from concourse.bass_utils import run_bass_kernel_spmd
from concourse.masks import make_identity

NUM_USERS = 30000
NUM_ITEMS = 70000
N = 100000
D = 128
NL = 3
BATCH = 16384
NCORE = 8
SHARD = N // NCORE            # 12500
RB = 256                      # dest rows per block
NB = (SHARD + RB - 1) // RB   # 49 blocks/core
RS = 32768                    # range size (int16 index window)
NR = 4                        # ranges
GMAX = 24                     # max chunks per dma_gather call
BSH = BATCH // NCORE          # 2048
EPS = 1e-12
SLOPE = 0.01

f32 = mybir.dt.float32
f32r = mybir.dt.float32r
i16 = mybir.dt.int16

_cache = {}

# debug bisection: 0=consts only, 1=+spmm, 2=+dense, 3=+allgather(1 layer),
# 4=+all layers, 5=full (default)
STAGE = int(os.environ.get("KSTAGE", "5"))
NLAYERS = int(os.environ.get("KNL", str(NL)))
KGROUPS = int(os.environ.get("KGROUPS", "100000"))
KANY = os.environ.get("KANY", "1") == "1"
KREPEAT = int(os.environ.get("KREPEAT", "1"))
KONLY = os.environ.get("KONLY", "")


def _build(meta):
    (NCH, k2, first_r, groups, chunk_info, NFB, fin_bounds) = meta
    nc = bacc.Bacc(num_devices=NCORE)

    feat0 = nc.dram_tensor("feat0", [N, D], f32, kind="ExternalInput")
    f0t = nc.dram_tensor("f0t", [D, SHARD], f32, kind="ExternalInput")
    eidx = nc.dram_tensor("eidx", [128, NCH * 8], i16, kind="ExternalInput")
    erow = nc.dram_tensor("erow", [128, NCH], f32, kind="ExternalInput")
    evals = nc.dram_tensor("evals", [128, NCH], f32, kind="ExternalInput")
    wlin = nc.dram_tensor("wlin", [D, NL * D], f32, kind="ExternalInput")
    wint = nc.dram_tensor("wint", [D, NL * D], f32, kind="ExternalInput")
    biasc = nc.dram_tensor("biasc", [D, NL], f32, kind="ExternalInput")
    uidx = nc.dram_tensor("uidx", [128, NFB * 8], i16, kind="ExternalInput")
    iidx = nc.dram_tensor("iidx", [128, NFB * 8], i16, kind="ExternalInput")
    score = nc.dram_tensor("score", [128, NFB], f32, kind="ExternalOutput")

    add = mybir.AluOpType.add
    mult = mybir.AluOpType.mult
    is_equal = mybir.AluOpType.is_equal
    maxop = mybir.AluOpType.max
    AF = mybir.ActivationFunctionType

    with tile.TileContext(nc) as tc:
        with (
            tc.tile_pool(name="const", bufs=1) as cp,
            tc.tile_pool(name="ft", bufs=1) as ftp,
            tc.tile_pool(name="g", bufs=2) as gp,
            tc.tile_pool(name="fin", bufs=1) as fp_,
            tc.tile_pool(name="ot", bufs=4) as otp,
            tc.tile_pool(name="sb", bufs=3) as sbp,
            tc.tile_pool(name="plx", bufs=2, space="PSUM") as plx,
            tc.tile_pool(name="py", bufs=2, space="PSUM") as pyp,
            tc.tile_pool(name="pn", bufs=1, space="PSUM") as pnp,
            tc.tile_pool(name="pbc", bufs=1, space="PSUM") as pbc,
            tc.tile_pool(name="ptp", bufs=2, space="PSUM") as ptp,
            tc.tile_pool(name="dram", bufs=1, space="DRAM") as dp,
        ):
            # ---------- constants ----------
            iota = cp.tile([128, RB], f32)
            nc.gpsimd.iota(iota[:], pattern=[[1, RB]], base=0,
                           channel_multiplier=0,
                           allow_small_or_imprecise_dtypes=True)
            ident = cp.tile([128, 128], f32)
            make_identity(nc, ident[:])
            ones_f = cp.tile([128, 1], f32)
            nc.vector.memset(ones_f[:], 1.0)
            ones_r = cp.tile([128, 1], f32r)
            nc.vector.tensor_copy(ones_r[:], ones_f[:])
            onesrow_f = cp.tile([1, 128], f32)
            nc.vector.memset(onesrow_f[:], 1.0)
            onesrow_r = cp.tile([1, 128], f32r)
            nc.vector.tensor_copy(onesrow_r[:], onesrow_f[:])

            wlin_f = cp.tile([128, NL * 128], f32)
            nc.sync.dma_start(out=wlin_f[:], in_=wlin[:])
            wlin_r = cp.tile([128, NL * 128], f32r)
            nc.vector.tensor_copy(wlin_r[:], wlin_f[:])
            wint_f = cp.tile([128, NL * 128], f32)
            nc.sync.dma_start(out=wint_f[:], in_=wint[:])
            wint_r = cp.tile([128, NL * 128], f32r)
            nc.vector.tensor_copy(wint_r[:], wint_f[:])
            bias_sb = cp.tile([128, NL], f32)
            nc.sync.dma_start(out=bias_sb[:], in_=biasc[:])

            eidx_sb = cp.tile([128, NCH * 8], i16)
            nc.sync.dma_start(out=eidx_sb[:], in_=eidx[:])
            erow_sb = cp.tile([128, NCH], f32)
            nc.sync.dma_start(out=erow_sb[:], in_=erow[:])
            evals_sb = cp.tile([128, NCH], f32)
            nc.sync.dma_start(out=evals_sb[:], in_=evals[:])
            uidx_sb = cp.tile([128, NFB * 8], i16)
            nc.sync.dma_start(out=uidx_sb[:], in_=uidx[:])
            iidx_sb = cp.tile([128, NFB * 8], i16)
            nc.sync.dma_start(out=iidx_sb[:], in_=iidx[:])

            fta = ftp.tile([128, NB * RB], f32, tag="fta")
            ftb = ftp.tile([128, NB * RB], f32, tag="ftb")
            nc.vector.memset(fta[:, SHARD:], 0.0)
            nc.sync.dma_start(out=fta[:, :SHARD], in_=f0t[:])

            fshard = dp.tile([SHARD, D], f32)
            agreps = [
                [dp.tile([N, D], f32, name=f"ag{r}_{i}", tag=f"ag{r}_{i}",
                         addr_space="Shared")
                 for i in range(NL)]
                for r in range(KREPEAT)
            ]
            ags = agreps[0]

            n_layers = NLAYERS if STAGE >= 4 else (1 if STAGE >= 1 else 0)
            for rep_l in range(KREPEAT * n_layers):
                rep, l = divmod(rep_l, n_layers)
                ftin = fta if l % 2 == 0 else ftb
                ftout = ftb if l % 2 == 0 else fta
                ags = agreps[rep]
                src = feat0 if l == 0 else ags[l - 1]

                # ---------- SpMM sweep (range-major chunk order) ----------
                cur_lx = None
                for gi, (gr, q0, gn) in enumerate(groups[:KGROUPS]):
                    gt = gp.tile([128, GMAX * 128], f32r, tag="g",
                                 name=f"g{rep}_{l}_{gi}")
                    nc.gpsimd.dma_gather(
                        gt[:, :gn * 128].rearrange("p (c d) -> p c d", d=128),
                        src[gr * RS:, :].bitcast(f32r),
                        eidx_sb[:, q0 * 8:(q0 + gn) * 8],
                        gn * 128, gn * 128, 128,
                        single_packet=False,
                    )
                    if KONLY == "gather":
                        continue
                    for q in range(q0, q0 + gn):
                        r_q, b_q, is_first, is_last, loc = chunk_info[q]
                        o = otp.tile([128, RB], f32r, tag="o",
                                     name=f"o{rep}_{l}_{q}")
                        ts_eng = nc.gpsimd if (KANY and q % 3 == 2) \
                            else nc.vector
                        ts_eng.tensor_scalar(
                            out=o[:], in0=iota[:],
                            scalar1=erow_sb[:, q:q + 1],
                            scalar2=evals_sb[:, q:q + 1],
                            op0=is_equal, op1=mult,
                        )
                        if KONLY == "ob":
                            continue
                        if is_first:
                            cur_lx = plx.tile([128, RB], f32, tag="lx",
                                              name=f"lx{rep}_{l}_{q}")
                        nc.tensor.matmul(
                            out=cur_lx[:],
                            lhsT=gt[:, ts(loc, 128)],
                            rhs=o[:],
                            start=is_first, stop=is_last,
                        )
                        if is_last and KONLY != "nof":
                            dst = ftout[:, ts(b_q, RB)]
                            if first_r[b_q] == r_q:
                                nc.vector.tensor_copy(dst, cur_lx[:])
                            else:
                                nc.vector.tensor_tensor(
                                    out=dst, in0=dst, in1=cur_lx[:], op=add)

                # ---------- dense phase ----------
                for b in range(NB if (STAGE >= 2 and not KONLY) else 0):
                    lxs = ftout[:, ts(b, RB)]
                    fin_ = ftin[:, ts(b, RB)]
                    pre1 = sbp.tile([128, RB], f32r, tag="pre1")
                    nc.vector.tensor_tensor(out=pre1[:], in0=lxs, in1=fin_,
                                            op=add)
                    pre2 = sbp.tile([128, RB], f32r, tag="pre2")
                    nc.vector.tensor_tensor(out=pre2[:], in0=lxs, in1=fin_,
                                            op=mult)
                    y = pyp.tile([128, RB], f32, tag="y")
                    nc.tensor.matmul(out=y[:], lhsT=wlin_r[:, ts(l, 128)],
                                     rhs=pre1[:], start=True, stop=False)
                    nc.tensor.matmul(out=y[:], lhsT=wint_r[:, ts(l, 128)],
                                     rhs=pre2[:], start=False, stop=True)
                    ya = sbp.tile([128, RB], f32, tag="ya")
                    nc.scalar.activation(out=ya[:], in_=y[:], func=AF.Lrelu,
                                         bias=bias_sb[:, l:l + 1], scale=1.0,
                                         alpha=SLOPE)
                    sq = sbp.tile([128, RB], f32r, tag="sq")
                    nc.vector.tensor_tensor(out=sq[:], in0=ya[:], in1=ya[:],
                                            op=mult)
                    nsq = pnp.tile([1, RB], f32, tag="nsq")
                    nc.tensor.matmul(out=nsq[:], lhsT=ones_r[:], rhs=sq[:],
                                     start=True, stop=True)
                    rt = sbp.tile([1, RB], f32, tag="rt")
                    nc.scalar.activation(out=rt[:], in_=nsq[:], func=AF.Sqrt)
                    rtm = sbp.tile([1, RB], f32, tag="rtm")
                    nc.vector.tensor_scalar(out=rtm[:], in0=rt[:],
                                            scalar1=EPS, scalar2=None,
                                            op0=maxop)
                    inv = sbp.tile([1, RB], f32r, tag="inv")
                    with nc.allow_low_precision(reason="f32r broadcast input"):
                        nc.vector.reciprocal(inv[:], rtm[:])
                    bc = pbc.tile([128, RB], f32, tag="bc")
                    nc.tensor.matmul(out=bc[:], lhsT=onesrow_r[:], rhs=inv[:],
                                     start=True, stop=True)
                    nc.vector.tensor_tensor(out=ftout[:, ts(b, RB)],
                                            in0=ya[:], in1=bc[:], op=mult)
                    # transpose to normal layout, write shard
                    for h in range(2):
                        r0 = b * RB + h * 128
                        nr = min(128, SHARD - r0)
                        if nr <= 0:
                            break
                        tp = ptp.tile([128, 128], f32, tag="tp",
                                      name=f"tp{rep}_{l}_{b}_{h}")
                        nc.tensor.transpose(
                            out=tp[:], in_=ftout[:, r0:r0 + 128],
                            identity=ident[:])
                        cpo = sbp.tile([128, 128], f32, tag="cpo")
                        nc.vector.tensor_copy(cpo[:], tp[:])
                        nc.sync.dma_start(out=fshard[r0:r0 + nr, :],
                                          in_=cpo[:nr, :])

                if STAGE >= 3:
                    nc.gpsimd.collective_compute(
                        "AllGather", mybir.AluOpType.bypass,
                        replica_groups=[list(range(NCORE))],
                        ins=[fshard.opt()], outs=[ags[l].opt()],
                    )

            # ---------- final gather + dot ----------
            acc = cp.tile([128, NFB], f32)
            if STAGE < 5:
                nc.vector.memset(acc[:], 0.0)
            fin_list = (([feat0] + ags) if STAGE >= 5 else []) * KREPEAT
            for li, srcf in enumerate(fin_list):
                ug = fp_.tile([128, NFB * 128], f32, tag="ug",
                              name=f"ug{li}")
                nc.gpsimd.dma_gather(
                    ug[:].rearrange("p (c d) -> p c d", d=128),
                    srcf[:],
                    uidx_sb[:],
                    NFB * 128, NFB * 128, 128,
                    single_packet=False,
                )
                ig = fp_.tile([128, NFB * 128], f32, tag="ig",
                              name=f"ig{li}")
                for (rr, c0, cn) in fin_bounds:
                    nc.gpsimd.dma_gather(
                        ig[:, c0 * 128:(c0 + cn) * 128].rearrange(
                            "p (c d) -> p c d", d=128),
                        srcf[rr * RS:, :],
                        iidx_sb[:, c0 * 8:(c0 + cn) * 8],
                        cn * 128, cn * 128, 128,
                        single_packet=False,
                    )
                nc.vector.tensor_tensor(out=ug[:], in0=ug[:], in1=ig[:],
                                        op=mult)
                sc = sbp.tile([128, NFB], f32, tag="sc")
                nc.vector.tensor_reduce(
                    out=sc[:],
                    in_=ug[:].rearrange("p (c d) -> p c d", d=128),
                    axis=mybir.AxisListType.X, op=add)
                if li % (NL + 1) == 0:
                    nc.vector.tensor_copy(acc[:], sc[:])
                else:
                    nc.vector.tensor_tensor(out=acc[:], in0=acc[:],
                                            in1=sc[:], op=add)
            nc.sync.dma_start(out=score[:], in_=acc[:])

    nc.compile()
    return nc


def _pack_inputs(userIdx, itemIdx, rows, cols, vals, uEmbd, iEmbd,
                 Wlin, blin, Wint, bint):
    rows = np.asarray(rows, dtype=np.int64)
    cols = np.asarray(cols, dtype=np.int64)
    vals = np.asarray(vals, dtype=np.float32)
    userIdx = np.asarray(userIdx, dtype=np.int64)
    itemIdx = np.asarray(itemIdx, dtype=np.int64)

    feat0 = np.ascontiguousarray(
        np.concatenate([np.asarray(uEmbd, np.float32),
                        np.asarray(iEmbd, np.float32)], axis=0))

    # ---- edge bucketing: (core, range, block) ----
    core = rows // SHARD
    local = rows - core * SHARD
    blk = local // RB
    rowl = (local - blk * RB).astype(np.float32)
    rng = cols // RS
    col_local = (cols - rng * RS).astype(np.int16)

    bkey = ((core * NR + rng) * NB + blk).astype(np.int64)
    order = np.argsort(bkey, kind="stable")
    bkey_s = bkey[order]
    counts = np.bincount(bkey_s, minlength=NCORE * NR * NB)
    counts = counts.reshape(NCORE, NR, NB)
    k2 = np.ceil(counts.max(axis=0) / 128).astype(np.int64)  # [NR, NB]

    # chunk layout (range-major), shared by all cores
    chunk_base = np.zeros((NR, NB), dtype=np.int64)
    nch = 0
    chunk_info = []          # per chunk: (r, b, is_first, is_last, loc)
    groups = []              # (r, q0, n)
    first_r = [None] * NB
    for r in range(NR):
        for b in range(NB):
            k = int(k2[r, b])
            if k == 0:
                continue
            if first_r[b] is None:
                first_r[b] = r
            chunk_base[r, b] = nch
            for j in range(k):
                chunk_info.append([r, b, j == 0, j == k - 1, 0])
            nch += k
    NCH = nch
    # gather groups: consecutive chunks, same range, <= GMAX
    q = 0
    while q < NCH:
        r = chunk_info[q][0]
        n = 1
        while (q + n < NCH and n < GMAX and chunk_info[q + n][0] == r):
            n += 1
        groups.append((r, q, n))
        for j in range(n):
            chunk_info[q + j][4] = j
        q += n
    chunk_info = [tuple(x) for x in chunk_info]

    # ---- scatter edges into slots ----
    starts = np.zeros(NCORE * NR * NB, dtype=np.int64)
    np.cumsum(counts.reshape(-1)[:-1], out=starts[1:])
    pos = np.arange(len(bkey_s), dtype=np.int64) - starts[bkey_s]
    core_s = core[order]
    rng_s = rng[order]
    blk_s = blk[order]
    q_of_edge = chunk_base[rng_s, blk_s] + pos // 128
    p_of_edge = pos % 128

    eidx_arr = np.zeros((NCORE, 16, NCH * 8), dtype=np.int16)
    erow_arr = np.zeros((NCORE, 128, NCH), dtype=np.float32)
    eval_arr = np.zeros((NCORE, 128, NCH), dtype=np.float32)
    eidx_arr[core_s, p_of_edge % 16, q_of_edge * 8 + p_of_edge // 16] = \
        col_local[order]
    erow_arr[core_s, p_of_edge, q_of_edge] = rowl[order]
    eval_arr[core_s, p_of_edge, q_of_edge] = vals[order]

    # ---- weights ----
    wlin_h = np.ascontiguousarray(
        np.asarray(Wlin, np.float32).transpose(1, 0, 2).reshape(D, NL * D))
    wint_h = np.ascontiguousarray(
        np.asarray(Wint, np.float32).transpose(1, 0, 2).reshape(D, NL * D))
    biasc = np.ascontiguousarray(
        (np.asarray(blin, np.float32) + np.asarray(bint, np.float32)).T)

    # ---- final stage: bucket item rows by range ----
    irow = itemIdx + NUM_USERS
    ir = irow // RS
    # per-core bucketing with shared padded sizes
    nfb_counts = np.zeros((NCORE, NR), dtype=np.int64)
    perms = []
    for c in range(NCORE):
        sl = slice(c * BSH, (c + 1) * BSH)
        o = np.argsort(ir[sl], kind="stable")
        perms.append(o)
        nfb_counts[c] = np.bincount(ir[sl][o], minlength=NR)
    bucket_chunks = np.ceil(nfb_counts.max(axis=0) / 128).astype(np.int64)
    # drop empty buckets, build (range, chunk0, nchunks)
    fin_bounds = []
    c0 = 0
    for r in range(NR):
        n = int(bucket_chunks[r])
        if n == 0:
            continue
        fin_bounds.append((r, c0, n))
        c0 += n
    NFB = c0

    uidx_arr = np.zeros((NCORE, 16, NFB * 8), dtype=np.int16)
    iidx_arr = np.zeros((NCORE, 16, NFB * 8), dtype=np.int16)
    inv_perm = np.full((NCORE, NFB * 128), -1, dtype=np.int64)
    for c in range(NCORE):
        sl = slice(c * BSH, (c + 1) * BSH)
        o = perms[c]
        u_s = userIdx[sl][o]
        i_s = irow[sl][o]
        r_s = ir[sl][o]
        # slot for j-th sorted elem: bucket r -> slots [b0*128 ...]
        jpos = np.zeros(BSH, dtype=np.int64)
        for (r, b0, nchk) in fin_bounds:
            m = r_s == r
            jpos[m] = b0 * 128 + np.arange(int(m.sum()))
        uidx_arr[c, jpos % 16, (jpos // 128) * 8 + (jpos % 128) // 16] = \
            u_s.astype(np.int16)
        iidx_arr[c, jpos % 16, (jpos // 128) * 8 + (jpos % 128) // 16] = \
            (i_s - r_s * RS).astype(np.int16)
        inv_perm[c, jpos] = np.arange(c * BSH, (c + 1) * BSH)[o]

    meta = (NCH, k2, tuple(first_r), tuple(groups), tuple(chunk_info),
            NFB, tuple(fin_bounds))

    in_maps = []
    for c in range(NCORE):
        f0t = np.ascontiguousarray(feat0[c * SHARD:(c + 1) * SHARD].T)
        in_maps.append({
            "feat0": feat0,
            "f0t": f0t,
            "eidx": np.ascontiguousarray(np.tile(eidx_arr[c], (8, 1))),
            "erow": np.ascontiguousarray(erow_arr[c]),
            "evals": np.ascontiguousarray(eval_arr[c]),
            "wlin": wlin_h,
            "wint": wint_h,
            "biasc": biasc,
            "uidx": np.ascontiguousarray(np.tile(uidx_arr[c], (8, 1))),
            "iidx": np.ascontiguousarray(np.tile(iidx_arr[c], (8, 1))),
        })
    return meta, in_maps, inv_perm


def kernel(**inputs) -> np.ndarray:
    meta, in_maps, inv_perm = _pack_inputs(**inputs)
    key = (meta[0], meta[5], meta[3], meta[4], meta[6], tuple(meta[2]))
    if key not in _cache:
        _cache[key] = _build(meta)
    nc = _cache[key]
    res = run_bass_kernel_spmd(nc, in_maps, list(range(NCORE)))
    out = np.empty(BATCH, dtype=np.float32)
    NFB = meta[5]
    for c in range(NCORE):
        sc = res.results[c]["score"]  # [128, NFB]
        # slot j -> sc[j % 128, j // 128]
        vals_j = sc[np.arange(NFB * 128) % 128, np.arange(NFB * 128) // 128]
        valid = inv_perm[c] >= 0
        out[inv_perm[c][valid]] = vals_j[valid]
    return out
